# revision 1
# baseline (speedup 1.0000x reference)
"""Trainium2 Bass kernel for nn_InterfaceGraph (retrieval_knn).

Segment-restricted nearest neighbors between pos_a and pos_b (16384 x
16384 pairwise distances, block-diagonal over 64 sorted graphs), sharded
over 8 NeuronCores (8 graphs per core, slot-sorted by size so the SPMD
program's per-slot shapes stay tight).

Per 128-row tile of a graph block, one bf16 matmul (K=21: a bf16x3
split of 2*a.b - |b|^2, small terms accumulated first) writes the
negated-distance key into PSUM at full speed; VectorE max/max_index read
PSUM directly and produce the row min + first-occurrence argmin, exactly
matching fp32 argmin semantics to ~1-2 ulp (validated: zero flips vs the
fp32 reference on the target data).  |a|^2 is omitted: it is constant
along the scanned axis, so it cannot change the argmin.  Both directions
(a->b, b->a) are computed the same way.

Host does the O(N) epilogue: gather + norm (same arithmetic as the
reference), residue segment-max interface mask, mutation OR, concat.
"""

import numpy as np
import ml_dtypes

NCORES = 8
G = 64
GPC = G // NCORES
NUM_RESIDUES = 2048
CUTOFF = np.float32(10.0)
BIG = np.float32(2.0 ** 26)
K = 21            # 9 tier-2 + 6 tier-1 + 3 tier-0 cross rows + 3 |b|^2 rows

PROFILE = False
LAST_EXEC_NS = None

BF16 = ml_dtypes.bfloat16

_prog_cache = {}


def _round_up(x, m):
    return (x + m - 1) // m * m


def _install_ntff_hook():
    import sys
    import types
    if 'antenv.axon_hooks' in sys.modules:
        return
    from trn_agent_boot.trn_boot import _ntff_profile_via_ctypes
    hook = _ntff_profile_via_ctypes('/opt/axon/libaxon_pjrt.so')
    mod = types.ModuleType('antenv.axon_hooks')
    mod.get_axon_ntff_profile_hook = lambda: hook
    sys.modules['antenv.axon_hooks'] = mod


def _split3(v):
    """bf16x3 split: v ~= v1 + v2 + v3 with ~24-bit mantissa coverage."""
    v = v.astype(np.float32)
    v1 = v.astype(BF16).astype(np.float32)
    r = v - v1
    v2 = r.astype(BF16).astype(np.float32)
    v3 = (r - v2).astype(BF16).astype(np.float32)
    return v1, v2, v3


class _Geom:
    """Per-slot shapes shared by all cores (SPMD program is one program).

    Slot assignment is independent per side: A-side slots sort each
    core's graphs by na desc (tile count), B-side by nb desc, which
    keeps the cross-core per-slot maxima tight.
    """

    def __init__(self, na, nb):
        gid = (np.arange(NCORES * GPC).reshape(NCORES, GPC) // GPC) * GPC
        ordA = np.zeros((NCORES, GPC), dtype=np.int64)
        ordB = np.zeros((NCORES, GPC), dtype=np.int64)
        for c in range(NCORES):
            loc = np.arange(GPC)
            ordA[c] = loc[np.argsort(-na[c * GPC + loc], kind="stable")]
            ordB[c] = loc[np.argsort(-nb[c * GPC + loc], kind="stable")]
        self.graphA = gid + ordA               # [core, slot] -> graph id
        self.graphB = gid + ordB
        na_A = na[self.graphA]
        nb_A = nb[self.graphA]
        nb_B = nb[self.graphB]
        na_B = na[self.graphB]
        self.TA = [int(-(-na_A[:, s].max() // 128)) for s in range(GPC)]
        self.TB = [int(-(-nb_B[:, s].max() // 128)) for s in range(GPC)]
        self.WB = [int(max(8, _round_up(int(nb_A[:, s].max()), 4)))
                   for s in range(GPC)]
        self.WA = [int(max(8, _round_up(int(na_B[:, s].max()), 4)))
                   for s in range(GPC)]
        self.baseTA = np.concatenate([[0], np.cumsum(self.TA)]).astype(int)
        self.baseTB = np.concatenate([[0], np.cumsum(self.TB)]).astype(int)
        self.baseWB = np.concatenate([[0], np.cumsum(self.WB)]).astype(int)
        self.baseWA = np.concatenate([[0], np.cumsum(self.WA)]).astype(int)

    def key(self):
        return (tuple(self.TA), tuple(self.TB), tuple(self.WB), tuple(self.WA))


def _build_program(geom):
    from contextlib import ExitStack

    import concourse.bacc as bacc
    import concourse.mybir as mybir
    import concourse.tile as tile

    f32 = mybir.dt.float32
    bf16 = mybir.dt.bfloat16
    u32 = mybir.dt.uint32

    LA = int(geom.baseTA[-1]) * 128   # lhsA columns
    LB = int(geom.baseTB[-1]) * 128
    RB = int(geom.baseWB[-1])         # rhsB columns
    RA = int(geom.baseWA[-1])
    OA = int(geom.baseTA[-1]) * 8     # output columns, a-side
    OB = int(geom.baseTB[-1]) * 8

    nc = bacc.Bacc("TRN2", target_bir_lowering=False, debug=False,
                   enable_asserts=True, num_devices=NCORES)

    lhsA = nc.dram_tensor("lhsA", [K, LA], bf16, kind="ExternalInput").ap()
    rhsB = nc.dram_tensor("rhsB", [K, RB], bf16, kind="ExternalInput").ap()
    lhsB = nc.dram_tensor("lhsB", [K, LB], bf16, kind="ExternalInput").ap()
    rhsA = nc.dram_tensor("rhsA", [K, RA], bf16, kind="ExternalInput").ap()
    idxA = nc.dram_tensor("idxA", [128, OA], u32, kind="ExternalOutput").ap()
    idxB = nc.dram_tensor("idxB", [128, OB], u32, kind="ExternalOutput").ap()

    with tile.TileContext(nc) as tc:
        with ExitStack() as ctx:
            const = ctx.enter_context(tc.tile_pool(name="const", bufs=1))
            psum = ctx.enter_context(
                tc.tile_pool(name="psum", bufs=8, space="PSUM"))
            work = ctx.enter_context(tc.tile_pool(name="work", bufs=6))

            lhsA_sb = const.tile([K, LA], bf16, tag="lhsA")
            nc.sync.dma_start(lhsA_sb[:], lhsA[:])
            rhsB_sb = const.tile([K, RB], bf16, tag="rhsB")
            nc.sync.dma_start(rhsB_sb[:], rhsB[:])
            lhsB_sb = const.tile([K, LB], bf16, tag="lhsB")
            nc.sync.dma_start(lhsB_sb[:], lhsB[:])
            rhsA_sb = const.tile([K, RA], bf16, tag="rhsA")
            nc.sync.dma_start(rhsA_sb[:], rhsA[:])

            valA_sb = const.tile([128, OA], f32, tag="valA")
            idxA_sb = const.tile([128, OA], u32, tag="idxA")
            valB_sb = const.tile([128, OB], f32, tag="valB")
            idxB_sb = const.tile([128, OB], u32, tag="idxB")

            def side(lhs_sb, rhs_sb, T, baseT, W, baseW, val_sb, idx_sb):
                for s in range(GPC):
                    for t in range(T[s]):
                        kk = int(baseT[s]) + t
                        ps = psum.tile([128, W[s]], f32, tag="ps")
                        nc.tensor.matmul(
                            ps[:],
                            lhs_sb[:, kk * 128:(kk + 1) * 128],
                            rhs_sb[:, int(baseW[s]):int(baseW[s]) + W[s]],
                            start=True, stop=True)
                        # VectorE max/max_index read PSUM directly (measured
                        # same per-op cost as SBUF; skipping the ScalarE
                        # copy shortens each tile's dependency chain).
                        nc.vector.max(val_sb[:, kk * 8:(kk + 1) * 8], ps[:])
                        nc.vector.max_index(
                            idx_sb[:, kk * 8:(kk + 1) * 8],
                            val_sb[:, kk * 8:(kk + 1) * 8], ps[:])

            side(lhsA_sb, rhsB_sb, geom.TA, geom.baseTA,
                 geom.WB, geom.baseWB, valA_sb, idxA_sb)
            side(lhsB_sb, rhsA_sb, geom.TB, geom.baseTB,
                 geom.WA, geom.baseWA, valB_sb, idxB_sb)

            nc.sync.dma_start(idxA[:], idxA_sb[:])
            nc.sync.dma_start(idxB[:], idxB_sb[:])

    nc.compile()
    return nc


def _pack_side(pos_row, pos_col, starts_row, starts_col, graphs,
               T, baseT, W, baseW):
    """lhs/rhs bf16 packs for one core, one direction.

    Row side (stationary): coords doubled, bf16x3 split.
    Col side (moving): coords + |q|^2 split; key = 2 p.q - |q|^2.
    K-row order: tier-2 (smallest) first, tier-0 last.
    """
    LT = int(baseT[-1]) * 128
    RW = int(baseW[-1])
    lhs = np.zeros((K, LT), dtype=np.float32)
    rhs = np.zeros((K, RW), dtype=np.float32)
    # q-split rows: tier2 row 9, tier1 rows 15-16?  layout below:
    #  rows 0-8   : tier2 cross (c,x3) lhs a1,a2,a3 / rhs b3,b2,b1
    #  row  9     : tier2 -q3      (lhs -1, rhs q3)
    #  rows 10-15 : tier1 cross    lhs a1,a2 / rhs b2,b1
    #  row  16    : tier1 -q2
    #  rows 17-19 : tier0 cross    lhs a1 / rhs b1
    #  row  20    : tier0 -q1  (+BIG on padding)
    lhs[9, :] = -1.0
    lhs[16, :] = -1.0
    lhs[20, :] = -1.0
    rhs[20, :] = BIG  # padding columns lose every argmax
    for s in range(GPC):
        g = graphs[s]
        p = pos_row[starts_row[g]:starts_row[g + 1]]
        n = p.shape[0]
        lb = int(baseT[s]) * 128
        for c in range(3):
            a1, a2, a3 = _split3(np.float32(2.0) * p[:, c])
            lhs[0 + c * 3, lb:lb + n] = a1
            lhs[1 + c * 3, lb:lb + n] = a2
            lhs[2 + c * 3, lb:lb + n] = a3
            lhs[10 + c * 2, lb:lb + n] = a1
            lhs[11 + c * 2, lb:lb + n] = a2
            lhs[17 + c, lb:lb + n] = a1
        # padding rows: zero coords, and kill the -1 rows so pad rows
        # read 0 - (-BIG)?  (pad rows' outputs are discarded anyway)

        q = pos_col[starts_col[g]:starts_col[g + 1]]
        m = q.shape[0]
        rb = int(baseW[s])
        qq = (q[:, 0] * q[:, 0] + q[:, 1] * q[:, 1]) + q[:, 2] * q[:, 2]
        q1, q2, q3 = _split3(qq)
        for c in range(3):
            b1, b2, b3 = _split3(q[:, c])
            rhs[0 + c * 3, rb:rb + m] = b3
            rhs[1 + c * 3, rb:rb + m] = b2
            rhs[2 + c * 3, rb:rb + m] = b1
            rhs[10 + c * 2, rb:rb + m] = b2
            rhs[11 + c * 2, rb:rb + m] = b1
            rhs[17 + c, rb:rb + m] = b1
        rhs[9, rb:rb + m] = q3
        rhs[16, rb:rb + m] = q2
        rhs[20, rb:rb + m] = q1
    return lhs.astype(BF16), rhs.astype(BF16)


def _unpack_side(res_idx, starts_row, starts_col, graphs, baseT, idx_full):
    for s in range(GPC):
        g = graphs[s]
        n = starts_row[g + 1] - starts_row[g]
        for t in range((n + 127) // 128):
            rows = min(128, n - t * 128)
            kk = int(baseT[s]) + t
            loc = res_idx[:rows, kk * 8].astype(np.int64)
            atoms = starts_row[g] + t * 128 + np.arange(rows)
            idx_full[atoms] = starts_col[g] + loc


def kernel(pos_a, pos_b, node2graph_a, node2graph_b,
           atom2residue_a, atom2residue_b, is_mutation):
    global LAST_EXEC_NS

    from concourse.bass_utils import run_bass_kernel_spmd

    pos_a = np.asarray(pos_a, dtype=np.float32)
    pos_b = np.asarray(pos_b, dtype=np.float32)
    node2graph_a = np.asarray(node2graph_a)
    node2graph_b = np.asarray(node2graph_b)
    atom2residue_a = np.asarray(atom2residue_a)
    atom2residue_b = np.asarray(atom2residue_b)
    is_mutation = np.asarray(is_mutation)

    Na = pos_a.shape[0]
    Nb = pos_b.shape[0]

    sa = np.searchsorted(node2graph_a, np.arange(G + 1)).astype(np.int64)
    sb = np.searchsorted(node2graph_b, np.arange(G + 1)).astype(np.int64)
    na = np.diff(sa)
    nb = np.diff(sb)
    assert na.min() > 0 and nb.min() > 0, "empty graph block not supported"

    geom = _Geom(na, nb)
    key = geom.key()
    if key not in _prog_cache:
        _prog_cache[key] = _build_program(geom)
    nc = _prog_cache[key]

    in_maps = []
    for c in range(NCORES):
        lhsA, rhsB = _pack_side(pos_a, pos_b, sa, sb, geom.graphA[c],
                                geom.TA, geom.baseTA, geom.WB, geom.baseWB)
        lhsB, rhsA = _pack_side(pos_b, pos_a, sb, sa, geom.graphB[c],
                                geom.TB, geom.baseTB, geom.WA, geom.baseWA)
        in_maps.append({"lhsA": lhsA, "rhsB": rhsB,
                        "lhsB": lhsB, "rhsA": rhsA})

    if PROFILE:
        _install_ntff_hook()
    res = run_bass_kernel_spmd(nc, in_maps, list(range(NCORES)),
                               trace=bool(PROFILE))
    if PROFILE:
        LAST_EXEC_NS = res.exec_time_ns

    idx_a = np.zeros(Na, dtype=np.int64)
    idx_b = np.zeros(Nb, dtype=np.int64)
    for c in range(NCORES):
        _unpack_side(res.results[c]["idxA"], sa, sb, geom.graphA[c],
                     geom.baseTA, idx_a)
        _unpack_side(res.results[c]["idxB"], sb, sa, geom.graphB[c],
                     geom.baseTB, idx_b)

    da = pos_a - pos_b[idx_a]
    dist_a = np.sqrt((da[:, 0] * da[:, 0] + da[:, 1] * da[:, 1])
                     + da[:, 2] * da[:, 2])
    db = pos_b - pos_a[idx_b]
    dist_b = np.sqrt((db[:, 0] * db[:, 0] + db[:, 1] * db[:, 1])
                     + db[:, 2] * db[:, 2])

    def iface_mask(dist, atom2residue):
        is_if = (dist < CUTOFF).astype(np.int32)
        res_max = np.zeros(NUM_RESIDUES, dtype=np.int32)
        np.maximum.at(res_max, atom2residue, is_if)
        return res_max[atom2residue] > 0

    mask_a = iface_mask(dist_a, atom2residue_a)
    mask_b = iface_mask(dist_b, atom2residue_b)
    mask = np.concatenate([mask_a, mask_b]) | is_mutation.astype(bool)
    dists = np.concatenate([dist_a, dist_b]).astype(np.float32)
    return mask, dists



# revision 3
# speedup vs baseline: 1.3848x; 1.3848x over previous
"""Trainium2 Bass kernel for nn_InterfaceGraph (retrieval_knn).

Segment-restricted nearest neighbors between pos_a and pos_b (16384 x
16384 pairwise distances, block-diagonal over 64 sorted graphs), sharded
over 8 NeuronCores (8 graphs per core, slot-sorted by size so the SPMD
program's per-slot shapes stay tight).

Per 128-row tile of a graph block, one bf16 matmul (K=21: a bf16x3
split of 2*a.b - |b|^2, small terms accumulated first) writes the
negated-distance key into PSUM at full speed.  A single custom DVE
instruction (ARGMAX_PACK) then does the whole argmax in one pass over
PSUM: per element it clears the low 9 mantissa bits of the fp32 key and
ORs in the column index (from a constant iota-bits tensor), and a MAX
accumulator folds the packed values to one [128,1] result per tile.
Bit-packing makes float-max order the keys at 2^-14 relative
quantization with the index riding in the low bits, so the argmin index
is accum & 0x1FF.  Host does the O(N) epilogue: gather + norm, a
near-cutoff exact-recompute band (covers the 2^-14 quantization at the
10.0 interface threshold), residue segment-max mask, mutation OR.
"""

import numpy as np
import ml_dtypes

NCORES = 8
G = 64
GPC = G // NCORES
NUM_RESIDUES = 2048
CUTOFF = np.float32(10.0)
BIG = np.float32(2.0 ** 26)
K = 21            # 9 tier-2 + 6 tier-1 + 3 tier-0 cross rows + 3 |b|^2 rows
WMAX = 512        # PSUM bank width; index must fit the low 9 bits

PROFILE = False
LAST_EXEC_NS = None

BF16 = ml_dtypes.bfloat16
FLT_MAX = np.float32(3.4028235e38)
IDX_MASK = np.uint32(0x1FF)

_prog_cache = {}
_argmax_op = None


def _round_up(x, m):
    return (x + m - 1) // m * m


def _install_ntff_hook():
    import sys
    import types
    if 'antenv.axon_hooks' in sys.modules:
        return
    from trn_agent_boot.trn_boot import _ntff_profile_via_ctypes
    hook = _ntff_profile_via_ctypes('/opt/axon/libaxon_pjrt.so')
    mod = types.ModuleType('antenv.axon_hooks')
    mod.get_axon_ntff_profile_hook = lambda: hook
    sys.modules['antenv.axon_hooks'] = mod


def _get_argmax_op():
    """Register the one-pass packed-argmax custom DVE op.

    body = (Src0 ^ (Src0 & C0)) | Src1  with C0 = bits 0x1FF (a denormal)
    == (key & ~0x1FF) | iota_bits, folded with a MAX accumulator.
    For the (negative) distance keys, float-max over the packed values
    picks the quantized-max key; ties take the smallest index.
    """
    global _argmax_op
    if _argmax_op is not None:
        return _argmax_op

    import concourse.dve_ops as dve_ops
    from concourse.dve_ops import DveOp
    from concourse.dve_spec import (
        Spec, Src0, Src1, Bin, lower, _has_src1, AluOp, C0, maxx)
    from concourse.dve_uop import DveOpSpec

    name = "ARGMAX_PACK_F32_ANT"
    if name in dve_ops._SUB_OPCODE_FOR_NAME:
        _argmax_op = next(o for o in dve_ops.OPS if o.name == name)
        return _argmax_op

    def _ref(in0, in1, s0, s1, imm2):
        m = ~np.float32(s0).view(np.uint32)
        x = np.ascontiguousarray(in0.astype(np.float32)).view(np.uint32)
        i = np.ascontiguousarray(in1.astype(np.float32)).view(np.uint32)
        b = ((x & m) | i).view(np.float32)
        acc = np.maximum(
            b.reshape(b.shape[0], -1).max(axis=1, keepdims=True), -FLT_MAX)
        return b, acc

    low = Bin(AluOp.BITWISE_AND, Src0, C0)
    spec = Spec(
        body=Bin(AluOp.BITWISE_OR, Bin(AluOp.BITWISE_XOR, Src0, low), Src1),
        accum=maxx, reference=_ref)

    row = dve_ops._CUSTOM_DVE_ROW_BASE + len(dve_ops.OPS)
    assert row < 0x20
    shas = {}
    for ver in ("v3", "v4"):
        s = DveOpSpec(name=name, opcode=row, uops=lower(spec, ver=ver),
                      rd1_en=_has_src1(spec))
        shas[ver] = s.sha(ver)
    op = DveOp(name, spec, subdim=False, uops_sha=shas)
    dve_ops.OPS.append(op)
    dve_ops.CUSTOM_DVE_SPECS[name] = spec
    dve_ops._SUB_OPCODE_FOR_NAME[name] = row
    _argmax_op = op
    return op


MASK_CONST = float(np.uint32(0x1FF).view(np.float32))  # denormal, bits 0x1FF


def _split3(v):
    """bf16x3 split: v ~= v1 + v2 + v3 with ~24-bit mantissa coverage."""
    v = v.astype(np.float32)
    v1 = v.astype(BF16).astype(np.float32)
    r = v - v1
    v2 = r.astype(BF16).astype(np.float32)
    v3 = (r - v2).astype(BF16).astype(np.float32)
    return v1, v2, v3


class _Geom:
    """Per-slot shapes shared by all cores (SPMD program is one program).

    Slot assignment is independent per side: A-side slots sort each
    core's graphs by na desc (tile count), B-side by nb desc, which
    keeps the cross-core per-slot maxima tight.
    """

    def __init__(self, na, nb):
        gid = (np.arange(NCORES * GPC).reshape(NCORES, GPC) // GPC) * GPC
        ordA = np.zeros((NCORES, GPC), dtype=np.int64)
        ordB = np.zeros((NCORES, GPC), dtype=np.int64)
        for c in range(NCORES):
            loc = np.arange(GPC)
            ordA[c] = loc[np.argsort(-na[c * GPC + loc], kind="stable")]
            ordB[c] = loc[np.argsort(-nb[c * GPC + loc], kind="stable")]
        self.graphA = gid + ordA               # [core, slot] -> graph id
        self.graphB = gid + ordB
        na_A = na[self.graphA]
        nb_A = nb[self.graphA]
        nb_B = nb[self.graphB]
        na_B = na[self.graphB]
        self.TA = [int(-(-na_A[:, s].max() // 128)) for s in range(GPC)]
        self.TB = [int(-(-nb_B[:, s].max() // 128)) for s in range(GPC)]
        self.WB = [int(max(8, _round_up(int(nb_A[:, s].max()), 4)))
                   for s in range(GPC)]
        self.WA = [int(max(8, _round_up(int(na_B[:, s].max()), 4)))
                   for s in range(GPC)]
        assert max(self.WB) <= WMAX and max(self.WA) <= WMAX
        self.baseTA = np.concatenate([[0], np.cumsum(self.TA)]).astype(int)
        self.baseTB = np.concatenate([[0], np.cumsum(self.TB)]).astype(int)
        self.baseWB = np.concatenate([[0], np.cumsum(self.WB)]).astype(int)
        self.baseWA = np.concatenate([[0], np.cumsum(self.WA)]).astype(int)

    def key(self):
        return (tuple(self.TA), tuple(self.TB), tuple(self.WB), tuple(self.WA))


def _build_program(geom):
    from contextlib import ExitStack

    import concourse.bacc as bacc
    import concourse.mybir as mybir
    import concourse.tile as tile

    f32 = mybir.dt.float32
    bf16 = mybir.dt.bfloat16

    op_argmax = _get_argmax_op()

    LA = int(geom.baseTA[-1]) * 128   # lhsA columns
    LB = int(geom.baseTB[-1]) * 128
    RB = int(geom.baseWB[-1])         # rhsB columns
    RA = int(geom.baseWA[-1])
    NTA = int(geom.baseTA[-1])        # tile counts (= accum columns)
    NTB = int(geom.baseTB[-1])
    WM = max(max(geom.WB), max(geom.WA))

    nc = bacc.Bacc("TRN2", target_bir_lowering=False, debug=False,
                   enable_asserts=True, num_devices=NCORES)

    lhsA = nc.dram_tensor("lhsA", [K, LA], bf16, kind="ExternalInput").ap()
    rhsB = nc.dram_tensor("rhsB", [K, RB], bf16, kind="ExternalInput").ap()
    lhsB = nc.dram_tensor("lhsB", [K, LB], bf16, kind="ExternalInput").ap()
    rhsA = nc.dram_tensor("rhsA", [K, RA], bf16, kind="ExternalInput").ap()
    iota = nc.dram_tensor("iota", [128, WM], f32, kind="ExternalInput").ap()
    accA = nc.dram_tensor("accA", [128, NTA], f32, kind="ExternalOutput").ap()
    accB = nc.dram_tensor("accB", [128, NTB], f32, kind="ExternalOutput").ap()

    with tile.TileContext(nc) as tc:
        with ExitStack() as ctx:
            const = ctx.enter_context(tc.tile_pool(name="const", bufs=1))
            psum = ctx.enter_context(
                tc.tile_pool(name="psum", bufs=8, space="PSUM"))

            lhsA_sb = const.tile([K, LA], bf16, tag="lhsA")
            nc.sync.dma_start(lhsA_sb[:], lhsA[:])
            rhsB_sb = const.tile([K, RB], bf16, tag="rhsB")
            nc.sync.dma_start(rhsB_sb[:], rhsB[:])
            lhsB_sb = const.tile([K, LB], bf16, tag="lhsB")
            nc.sync.dma_start(lhsB_sb[:], lhsB[:])
            rhsA_sb = const.tile([K, RA], bf16, tag="rhsA")
            nc.sync.dma_start(rhsA_sb[:], rhsA[:])
            iota_sb = const.tile([128, WM], f32, tag="iota")
            nc.sync.dma_start(iota_sb[:], iota[:])

            accA_sb = const.tile([128, NTA], f32, tag="accA")
            accB_sb = const.tile([128, NTB], f32, tag="accB")

            def side(lhs_sb, rhs_sb, T, baseT, W, baseW, acc_sb):
                # One matmul + ONE fused DVE op per tile: the custom op
                # reads the PSUM keys once, packs (quantized key | col
                # index), and max-folds to acc_sb[:, kk].  out= is an
                # in-place write over the dying PSUM tile.
                for s in range(GPC):
                    for t in range(T[s]):
                        kk = int(baseT[s]) + t
                        ps = psum.tile([128, W[s]], f32, tag="ps")
                        nc.tensor.matmul(
                            ps[:],
                            lhs_sb[:, kk * 128:(kk + 1) * 128],
                            rhs_sb[:, int(baseW[s]):int(baseW[s]) + W[s]],
                            start=True, stop=True)
                        nc.vector._custom_dve(
                            op_argmax, out=ps[:],
                            accum_out=acc_sb[:, kk:kk + 1],
                            in0=ps[:], in1=iota_sb[:, :W[s]],
                            s0=MASK_CONST)

            side(lhsA_sb, rhsB_sb, geom.TA, geom.baseTA,
                 geom.WB, geom.baseWB, accA_sb)
            side(lhsB_sb, rhsA_sb, geom.TB, geom.baseTB,
                 geom.WA, geom.baseWA, accB_sb)

            nc.sync.dma_start(accA[:], accA_sb[:])
            nc.sync.dma_start(accB[:], accB_sb[:])

    nc.compile()
    return nc


def _pack_side(pos_row, pos_col, starts_row, starts_col, graphs,
               T, baseT, W, baseW):
    """lhs/rhs bf16 packs for one core, one direction.

    Row side (stationary): coords doubled, bf16x3 split.
    Col side (moving): coords + |q|^2 split; key = 2 p.q - |q|^2.
    K-row order: tier-2 (smallest) first, tier-0 last.
    """
    LT = int(baseT[-1]) * 128
    RW = int(baseW[-1])
    lhs = np.zeros((K, LT), dtype=np.float32)
    rhs = np.zeros((K, RW), dtype=np.float32)
    #  rows 0-8   : tier2 cross (c,x3) lhs a1,a2,a3 / rhs b3,b2,b1
    #  row  9     : tier2 -q3      (lhs -1, rhs q3)
    #  rows 10-15 : tier1 cross    lhs a1,a2 / rhs b2,b1
    #  row  16    : tier1 -q2
    #  rows 17-19 : tier0 cross    lhs a1 / rhs b1
    #  row  20    : tier0 -q1  (+BIG on padding)
    lhs[9, :] = -1.0
    lhs[16, :] = -1.0
    lhs[20, :] = -1.0
    rhs[20, :] = BIG  # padding columns lose every argmax
    for s in range(GPC):
        g = graphs[s]
        p = pos_row[starts_row[g]:starts_row[g + 1]]
        n = p.shape[0]
        lb = int(baseT[s]) * 128
        for c in range(3):
            a1, a2, a3 = _split3(np.float32(2.0) * p[:, c])
            lhs[0 + c * 3, lb:lb + n] = a1
            lhs[1 + c * 3, lb:lb + n] = a2
            lhs[2 + c * 3, lb:lb + n] = a3
            lhs[10 + c * 2, lb:lb + n] = a1
            lhs[11 + c * 2, lb:lb + n] = a2
            lhs[17 + c, lb:lb + n] = a1

        q = pos_col[starts_col[g]:starts_col[g + 1]]
        m = q.shape[0]
        rb = int(baseW[s])
        qq = (q[:, 0] * q[:, 0] + q[:, 1] * q[:, 1]) + q[:, 2] * q[:, 2]
        q1, q2, q3 = _split3(qq)
        for c in range(3):
            b1, b2, b3 = _split3(q[:, c])
            rhs[0 + c * 3, rb:rb + m] = b3
            rhs[1 + c * 3, rb:rb + m] = b2
            rhs[2 + c * 3, rb:rb + m] = b1
            rhs[10 + c * 2, rb:rb + m] = b2
            rhs[11 + c * 2, rb:rb + m] = b1
            rhs[17 + c, rb:rb + m] = b1
        rhs[9, rb:rb + m] = q3
        rhs[16, rb:rb + m] = q2
        rhs[20, rb:rb + m] = q1
    return lhs.astype(BF16), rhs.astype(BF16)


def _unpack_side(acc, starts_row, starts_col, graphs, baseT, idx_full):
    bits = np.ascontiguousarray(acc.astype(np.float32)).view(np.uint32)
    for s in range(GPC):
        g = graphs[s]
        n = starts_row[g + 1] - starts_row[g]
        for t in range((n + 127) // 128):
            rows = min(128, n - t * 128)
            kk = int(baseT[s]) + t
            loc = (bits[:rows, kk] & IDX_MASK).astype(np.int64)
            atoms = starts_row[g] + t * 128 + np.arange(rows)
            idx_full[atoms] = starts_col[g] + loc


def _fix_band(pos_row, pos_col, n2g_row, starts_col, dist, idx):
    """Exact recompute for rows whose dist lands near the 10.0 cutoff.

    The packed argmax quantizes keys at 2^-14 relative, so a selected
    neighbor can sit up to ~6e-5 relative above the true min; only rows
    within a hair of the interface threshold can flip the mask.  Redo
    those rows with the reference formula.
    """
    band = np.nonzero(np.abs(dist - CUTOFF) < np.float32(0.02))[0]
    for atom in band:
        g = int(n2g_row[atom])
        seg = pos_col[starts_col[g]:starts_col[g + 1]]
        p = pos_row[atom]
        d2 = ((p * p).sum() + (seg * seg).sum(axis=1)
              - np.float32(2.0) * (seg @ p))
        j = int(np.argmin(d2))
        idx[atom] = starts_col[g] + j
        da = p - seg[j]
        dist[atom] = np.sqrt((da[0] * da[0] + da[1] * da[1]) + da[2] * da[2])


def kernel(pos_a, pos_b, node2graph_a, node2graph_b,
           atom2residue_a, atom2residue_b, is_mutation):
    global LAST_EXEC_NS

    from concourse.bass_utils import run_bass_kernel_spmd

    pos_a = np.asarray(pos_a, dtype=np.float32)
    pos_b = np.asarray(pos_b, dtype=np.float32)
    node2graph_a = np.asarray(node2graph_a)
    node2graph_b = np.asarray(node2graph_b)
    atom2residue_a = np.asarray(atom2residue_a)
    atom2residue_b = np.asarray(atom2residue_b)
    is_mutation = np.asarray(is_mutation)

    Na = pos_a.shape[0]
    Nb = pos_b.shape[0]

    sa = np.searchsorted(node2graph_a, np.arange(G + 1)).astype(np.int64)
    sb = np.searchsorted(node2graph_b, np.arange(G + 1)).astype(np.int64)
    na = np.diff(sa)
    nb = np.diff(sb)
    assert na.min() > 0 and nb.min() > 0, "empty graph block not supported"

    geom = _Geom(na, nb)
    key = geom.key()
    if key not in _prog_cache:
        _prog_cache[key] = _build_program(geom)
    nc = _prog_cache[key]

    WM = max(max(geom.WB), max(geom.WA))
    iota_bits = np.arange(WM, dtype=np.uint32)[None, :].repeat(
        128, axis=0).view(np.float32)

    in_maps = []
    for c in range(NCORES):
        lhsA, rhsB = _pack_side(pos_a, pos_b, sa, sb, geom.graphA[c],
                                geom.TA, geom.baseTA, geom.WB, geom.baseWB)
        lhsB, rhsA = _pack_side(pos_b, pos_a, sb, sa, geom.graphB[c],
                                geom.TB, geom.baseTB, geom.WA, geom.baseWA)
        in_maps.append({"lhsA": lhsA, "rhsB": rhsB,
                        "lhsB": lhsB, "rhsA": rhsA, "iota": iota_bits})

    if PROFILE:
        _install_ntff_hook()
    res = run_bass_kernel_spmd(nc, in_maps, list(range(NCORES)),
                               trace=bool(PROFILE))
    if PROFILE:
        LAST_EXEC_NS = res.exec_time_ns

    idx_a = np.zeros(Na, dtype=np.int64)
    idx_b = np.zeros(Nb, dtype=np.int64)
    for c in range(NCORES):
        _unpack_side(res.results[c]["accA"], sa, sb, geom.graphA[c],
                     geom.baseTA, idx_a)
        _unpack_side(res.results[c]["accB"], sb, sa, geom.graphB[c],
                     geom.baseTB, idx_b)

    da = pos_a - pos_b[idx_a]
    dist_a = np.sqrt((da[:, 0] * da[:, 0] + da[:, 1] * da[:, 1])
                     + da[:, 2] * da[:, 2])
    db = pos_b - pos_a[idx_b]
    dist_b = np.sqrt((db[:, 0] * db[:, 0] + db[:, 1] * db[:, 1])
                     + db[:, 2] * db[:, 2])

    _fix_band(pos_a, pos_b, node2graph_a, sb, dist_a, idx_a)
    _fix_band(pos_b, pos_a, node2graph_b, sa, dist_b, idx_b)

    def iface_mask(dist, atom2residue):
        is_if = (dist < CUTOFF).astype(np.int32)
        res_max = np.zeros(NUM_RESIDUES, dtype=np.int32)
        np.maximum.at(res_max, atom2residue, is_if)
        return res_max[atom2residue] > 0

    mask_a = iface_mask(dist_a, atom2residue_a)
    mask_b = iface_mask(dist_b, atom2residue_b)
    mask = np.concatenate([mask_a, mask_b]) | is_mutation.astype(bool)
    dists = np.concatenate([dist_a, dist_b]).astype(np.float32)
    return mask, dists
